# revision 10
# baseline (speedup 1.0000x reference)
"""Distributed APPNP-stack kernel for one TRN2 chip (8 NeuronCores).

Pipeline: Linear(256->256) -> BatchNorm(batch stats) -> Linear(256->64)
-> 10 APPNP hops with GCN normalization -> log_softmax.

Sharding: nodes are partitioned contiguously across the 8 cores (12500
each). Each hop, every core publishes its shard of dinv*carry (bf16),
an AllGather makes the full table visible, and each core gathers the
source rows of the edges targeting its own nodes (dma_gather, int16
windows) and scatter-adds them with a one-hot-matmul into PSUM.

Host-side preprocessing is pure integer index work (CSR-style sort,
bucket/pad layout, degree counts). All float math runs on device.
"""

import hashlib
import inspect
import re

import ml_dtypes
import numpy as np

import concourse.bacc as bacc
import concourse.bass as bass
import concourse.mybir as mybir
from concourse.tile import TileContext
from concourse.bass_utils import run_bass_kernel_spmd

# ---------------------------------------------------------------- constants
N = 100000
E = 3200000
DIN = 256
HID = 256
DOUT = 64
KHOPS = 10
ALPHA = 0.1
BNEPS = 1e-5

NCORES = 8
NLOC = N // NCORES          # 12500 nodes owned per core
NB = (NLOC + 127) // 128    # 98 target blocks per core
NPAD = NB * 128             # 12544 padded local nodes
NREAL_LAST = NLOC - (NB - 1) * 128  # 84 real targets in the last block
NWIN = 4                    # int16 source windows (2 shards each)
WROWS = 2 * NPAD            # 25088 rows per window
SBLK = 7                    # target blocks per gather superblock
NSB = NB // SBLK            # 14
TW = 2 * DOUT               # padded table row width (256B bf16 stride)

F32 = mybir.dt.float32
BF16 = mybir.dt.bfloat16
I32 = mybir.dt.int32
I16 = mybir.dt.int16
AF = mybir.ActivationFunctionType
ALU = mybir.AluOpType
AX = mybir.AxisListType

_CACHE = {}
STAGE = "full"  # debug: front | pub | gather | full


def _patch_dma_gather():
    """Relax dma_gather's 256B elem-size assert to 128B (the real ucode
    constraint is the 256B *stride*, which we satisfy via elem_step)."""
    if getattr(bass.BassGpSimd.dma_gather, "_relaxed", False):
        return
    src = inspect.getsource(bass.BassGpSimd.dma_gather)
    src = src.replace("elem_size_bytes % 256 == 0", "elem_size_bytes % 128 == 0")
    src = "def dma_gather" + src.split("def dma_gather", 1)[1]
    src = re.sub(r"^    ", "", src, flags=re.M)
    ns = dict(bass.__dict__)
    exec(src, ns)
    ns["dma_gather"]._relaxed = True
    bass.BassGpSimd.dma_gather = ns["dma_gather"]


# ---------------------------------------------------------------- host prep
def _preprocess(edge_index):
    row = edge_index[0].astype(np.int64)
    col = edge_index[1].astype(np.int64)

    owner = col // NLOC
    lt = col - owner * NLOC
    blk = lt >> 7
    cloc = (lt & 127).astype(np.int32)
    gsrc = (row // NLOC) * NPAD + (row % NLOC)
    win = gsrc // WROWS
    lsrc = (gsrc - win * WROWS).astype(np.int32)

    key = (owner * NB + blk) * NWIN + win
    counts = np.bincount(key, minlength=NCORES * NB * NWIN).reshape(
        NCORES, NB, NWIN
    )
    order = np.argsort(key, kind="stable")
    lsrc_s = lsrc[order]
    cloc_s = cloc[order]
    starts = np.zeros(NCORES * NB * NWIN + 1, np.int64)
    np.cumsum(counts.ravel(), out=starts[1:])

    # uniform chunk count per (block, window): identical loop body on all
    # cores and all superblocks (required for the For_i sem-reset structure)
    CU = int(-(-counts.max() // 128))
    TOTC = NB * NWIN * CU
    NI16U = 8 * SBLK * CU                      # idx cols per (sb, w)
    TOT16 = NSB * NWIN * NI16U

    # per-core tables
    idx_all = np.zeros((NCORES, 128, TOT16), np.int16)
    colloc_all = np.full((NCORES, 128, TOTC), -1.0, np.float32)
    for c in range(NCORES):
        for b in range(NB):
            for w in range(NWIN):
                s0 = starts[(c * NB + b) * NWIN + w]
                s1 = starts[(c * NB + b) * NWIN + w + 1]
                cnt = int(s1 - s0)
                flatc = np.full(CU * 128, -1.0, np.float32)
                flatc[:cnt] = cloc_s[s0:s1]
                gcol = (b * NWIN + w) * CU
                colloc_all[c, :, gcol : gcol + CU] = flatc.reshape(CU, 128).T
        for sb in range(NSB):
            for w in range(NWIN):
                ni = SBLK * CU * 128
                flat = np.zeros(ni, np.int16)
                for i, b in enumerate(range(sb * SBLK, (sb + 1) * SBLK)):
                    s0 = starts[(c * NB + b) * NWIN + w]
                    s1 = starts[(c * NB + b) * NWIN + w + 1]
                    off = i * CU * 128
                    flat[off : off + (s1 - s0)] = lsrc_s[s0:s1]
                wrapped = flat.reshape(-1, 16).T          # [16, ni/16]
                c0 = (sb * NWIN + w) * NI16U
                idx_all[c, :, c0 : c0 + ni // 16] = np.tile(wrapped, (8, 1))

    deg = np.bincount(col, minlength=N).astype(np.int32) + 1
    deg_t = np.ones((NCORES, 128, NB), np.int32)
    for c in range(NCORES):
        dl = np.ones(NPAD, np.int32)
        dl[:NLOC] = deg[c * NLOC : (c + 1) * NLOC]
        deg_t[c] = dl.reshape(NB, 128).T

    return dict(
        CU=CU, TOTC=TOTC, TOT16=TOT16, NI16U=NI16U,
        idx_all=idx_all, colloc_all=colloc_all, deg_t=deg_t,
    )


# ---------------------------------------------------------------- program
def _build(plan):
    _patch_dma_gather()
    CU = plan["CU"]
    TOTC = plan["TOTC"]
    TOT16 = plan["TOT16"]
    NI16U = plan["NI16U"]
    NIU = SBLK * CU * 128                      # idxs per (sb, w) gather
    RG = [list(range(NCORES))]

    nc = bacc.Bacc("TRN2", debug=False, num_devices=NCORES)

    x_d = nc.dram_tensor("x", [NPAD, DIN], F32, kind="ExternalInput")
    w1_d = nc.dram_tensor("w1", [DIN, HID], F32, kind="ExternalInput")
    w2_d = nc.dram_tensor("w2", [HID, DOUT], F32, kind="ExternalInput")
    b1_d = nc.dram_tensor("b1h", [128, 2], F32, kind="ExternalInput")
    gm_d = nc.dram_tensor("gammah", [128, 2], F32, kind="ExternalInput")
    bt_d = nc.dram_tensor("betah", [128, 2], F32, kind="ExternalInput")
    b2_d = nc.dram_tensor("b2c", [DOUT, 1], F32, kind="ExternalInput")
    deg_d = nc.dram_tensor("degt", [128, NB], I32, kind="ExternalInput")
    iota_d = nc.dram_tensor("iotab", [128, 128], BF16, kind="ExternalInput")
    id128_d = nc.dram_tensor("id128", [128, 128], F32, kind="ExternalInput")
    id64_d = nc.dram_tensor("id64", [64, 64], F32, kind="ExternalInput")
    idx_d = nc.dram_tensor("idx", [128, TOT16], I16, kind="ExternalInput")
    cl_d = nc.dram_tensor("colloc", [128, TOTC], F32, kind="ExternalInput")
    out_d = nc.dram_tensor("out", [NPAD, DOUT], F32, kind="ExternalOutput")
    emb_d = nc.dram_tensor("emb", [NPAD, DOUT], F32, kind="ExternalOutput")

    agin_d = nc.dram_tensor("agin", [NPAD, TW], BF16)
    agfull_d = nc.dram_tensor("agfull", [NCORES * NPAD, TW], BF16,
                              addr_space="Shared")
    stin_d = nc.dram_tensor("stin", [128, 4], F32)
    stout_d = nc.dram_tensor("stout", [128, 4], F32, addr_space="Shared")

    with TileContext(nc) as tc:
        with (
            tc.tile_pool(name="persist", bufs=1) as PP,
            tc.tile_pool(name="ppsum", bufs=2, space="PSUM") as PPS,
        ):
            # ---------------- persistent tiles
            carry = PP.tile([128, NB * DOUT], F32)
            adz0 = PP.tile([128, NB * DOUT], F32)
            colloc = PP.tile([128, TOTC], F32)
            iotab = PP.tile([128, 128], BF16)
            id128 = PP.tile([128, 128], F32)
            id64 = PP.tile([64, 64], F32)
            w1k = [PP.tile([128, HID], F32, tag=f"w1_{k}", name=f"w1_{k}") for k in range(2)]
            w2k = [PP.tile([128, DOUT], F32, tag=f"w2_{k}", name=f"w2_{k}") for k in range(2)]
            w2p = [PP.tile([128, DOUT], F32, tag=f"w2p_{k}", name=f"w2p_{k}") for k in range(2)]
            b1t = PP.tile([128, 2], F32)
            gmt = PP.tile([128, 2], F32)
            btt = PP.tile([128, 2], F32)
            b2c = PP.tile([DOUT, 1], F32)
            b2tot = PP.tile([DOUT, 1], F32)
            degf = PP.tile([128, NB], F32)
            sqdeg = PP.tile([128, NB], F32)
            dinv = PP.tile([128, NB], F32)
            adinv = PP.tile([128, NB], F32)
            dinv1s = PP.tile([128, NB], F32)
            dinv2s = PP.tile([128, NB], F32)
            ones11 = PP.tile([1, 1], F32)
            epsb = PP.tile([128, 1], F32)
            hs = [PP.tile([128, NB], F32, tag=f"hs{m}", name=f"hs{m}") for m in range(2)]
            hq = [PP.tile([128, NB], F32, tag=f"hq{m}", name=f"hq{m}") for m in range(2)]
            stats = PP.tile([128, 4], F32)
            statsg = PP.tile([128, 4], F32)

            nc.sync.dma_start(out=colloc[:], in_=cl_d[:, :])
            nc.sync.dma_start(out=iotab[:], in_=iota_d[:, :])
            nc.sync.dma_start(out=id128[:], in_=id128_d[:, :])
            nc.sync.dma_start(out=id64[:], in_=id64_d[:, :])
            nc.sync.dma_start(out=w1k[0][:], in_=w1_d[0:128, :])
            nc.sync.dma_start(out=w1k[1][:], in_=w1_d[128:256, :])
            nc.sync.dma_start(out=w2k[0][:], in_=w2_d[0:128, :])
            nc.sync.dma_start(out=w2k[1][:], in_=w2_d[128:256, :])
            nc.sync.dma_start(out=b1t[:], in_=b1_d[:, :])
            nc.sync.dma_start(out=gmt[:], in_=gm_d[:, :])
            nc.sync.dma_start(out=btt[:], in_=bt_d[:, :])
            nc.sync.dma_start(out=b2c[:], in_=b2_d[:, :])
            dg_i = PP.tile([128, NB], I32)
            nc.sync.dma_start(out=dg_i[:], in_=deg_d[:, :])
            nc.vector.memset(ones11[:], 1.0)
            nc.vector.memset(epsb[:], BNEPS)

            # degree-derived per-node scalars
            nc.vector.tensor_copy(out=degf[:], in_=dg_i[:])
            nc.scalar.sqrt(sqdeg[:], degf[:])
            nc.vector.reciprocal(dinv[:], sqdeg[:])
            nc.vector.tensor_scalar(out=adinv[:], in0=dinv[:], scalar1=ALPHA,
                                    scalar2=None, op0=ALU.mult)
            nc.vector.tensor_scalar(out=dinv1s[:], in0=dinv[:],
                                    scalar1=1.0 - ALPHA, scalar2=None,
                                    op0=ALU.mult)
            nc.vector.tensor_mul(dinv2s[:], dinv[:], dinv1s[:])

            # ---------------- front: x @ W1 (+stats) -> BN fold -> @W2
            with (
                tc.tile_pool(name="fbig", bufs=1) as FB,
                tc.tile_pool(name="fwork", bufs=3) as FW,
                tc.tile_pool(name="fpsum", bufs=2, space="PSUM") as FPS,
            ):
                hT = [FB.tile([128, NPAD], F32, tag=f"hT{m}", name=f"hT{m}") for m in range(2)]
                for i in range(NB):
                    xt = FW.tile([128, DIN], F32, tag="xt")
                    nc.sync.dma_start(out=xt[:], in_=x_d[i * 128:(i + 1) * 128, :])
                    xT = []
                    for k in range(2):
                        tp = FPS.tile([128, 128], F32, tag="tp")
                        nc.tensor.transpose(out=tp[:],
                                            in_=xt[:, k * 128:(k + 1) * 128],
                                            identity=id128[:])
                        xs = FW.tile([128, 128], F32, tag=f"xT{k}")
                        nc.vector.tensor_copy(out=xs[:], in_=tp[:])
                        xT.append(xs)
                    for m in range(2):
                        hp = FPS.tile([128, 128], F32, tag="hp")
                        for k in range(2):
                            nc.tensor.matmul(hp[:],
                                             lhsT=w1k[k][:, m * 128:(m + 1) * 128],
                                             rhs=xT[k][:],
                                             start=(k == 0), stop=(k == 1))
                        dst = hT[m][:, i * 128:(i + 1) * 128]
                        if i < NB - 1:
                            nc.scalar.activation(dst, hp[:], AF.Identity,
                                                 bias=b1t[:, m:m + 1],
                                                 accum_out=hs[m][:, i:i + 1])
                            sqs = FW.tile([128, 128], F32, tag="sqs")
                            nc.scalar.activation(sqs[:], dst, AF.Square,
                                                 accum_out=hq[m][:, i:i + 1])
                        else:
                            nr = NREAL_LAST
                            nc.scalar.activation(dst[:, :nr], hp[:, :nr],
                                                 AF.Identity,
                                                 bias=b1t[:, m:m + 1],
                                                 accum_out=hs[m][:, i:i + 1])
                            sqs = FW.tile([128, 128], F32, tag="sqs")
                            nc.scalar.activation(sqs[:, :nr], dst[:, :nr],
                                                 AF.Square,
                                                 accum_out=hq[m][:, i:i + 1])
                            nc.vector.tensor_copy(out=dst[:, nr:],
                                                  in_=hp[:, nr:])

                # global batch stats
                nc.vector.reduce_sum(stats[:, 0:1], hs[0][:], axis=AX.X)
                nc.vector.reduce_sum(stats[:, 1:2], hs[1][:], axis=AX.X)
                nc.vector.reduce_sum(stats[:, 2:3], hq[0][:], axis=AX.X)
                nc.vector.reduce_sum(stats[:, 3:4], hq[1][:], axis=AX.X)
                nc.sync.dma_start(out=stin_d[:, :], in_=stats[:])
                nc.gpsimd.collective_compute(
                    "AllReduce", ALU.add, replica_groups=RG,
                    ins=[stin_d.ap().opt()], outs=[stout_d.ap().opt()])
                nc.sync.dma_start(out=statsg[:], in_=stout_d[:, :])

                mean = FW.tile([128, 2], F32, tag="mean")
                ex2 = FW.tile([128, 2], F32, tag="ex2")
                var = FW.tile([128, 2], F32, tag="var")
                sd = FW.tile([128, 2], F32, tag="sd")
                sinv = FW.tile([128, 2], F32, tag="sinv")
                aa = FW.tile([128, 2], F32, tag="aa")
                cc = FW.tile([128, 2], F32, tag="cc")
                nc.vector.tensor_scalar(out=mean[:], in0=statsg[:, 0:2],
                                        scalar1=1.0 / N, scalar2=None,
                                        op0=ALU.mult)
                nc.vector.tensor_scalar(out=ex2[:], in0=statsg[:, 2:4],
                                        scalar1=1.0 / N, scalar2=None,
                                        op0=ALU.mult)
                msq = FW.tile([128, 2], F32, tag="msq")
                nc.vector.tensor_mul(msq[:], mean[:], mean[:])
                nc.vector.tensor_sub(var[:], ex2[:], msq[:])
                nc.scalar.activation(sd[:], var[:], AF.Sqrt, bias=epsb[:, 0:1])
                nc.vector.reciprocal(sinv[:], sd[:])
                nc.vector.tensor_mul(aa[:], sinv[:], gmt[:])
                mt_ = FW.tile([128, 2], F32, tag="mt_")
                nc.vector.tensor_mul(mt_[:], mean[:], aa[:])
                nc.vector.tensor_sub(cc[:], btt[:], mt_[:])
                for k in range(2):
                    nc.vector.tensor_scalar(out=w2p[k][:], in0=w2k[k][:],
                                            scalar1=aa[:, k:k + 1],
                                            scalar2=None, op0=ALU.mult)
                bb = FPS.tile([1, DOUT], F32, tag="tp")
                for k in range(2):
                    nc.tensor.matmul(bb[:], lhsT=cc[:, k:k + 1], rhs=w2k[k][:],
                                     start=(k == 0), stop=(k == 1))
                btmp = FW.tile([1, DOUT], F32, tag="btmp")
                nc.vector.tensor_copy(out=btmp[:], in_=bb[:])
                bcp = FPS.tile([DOUT, 1], F32, tag="hp")
                nc.tensor.matmul(bcp[:], lhsT=btmp[:], rhs=ones11[:],
                                 start=True, stop=True)
                nc.vector.tensor_add(b2tot[:], bcp[:], b2c[:])

                # z = hBN @ W2' ; store carry'_0 and alpha*dinv*z0
                for i in range(NB):
                    zp = FPS.tile([DOUT, 128], F32, tag="tp")
                    for k in range(2):
                        nc.tensor.matmul(zp[:], lhsT=w2p[k][:],
                                         rhs=hT[k][:, i * 128:(i + 1) * 128],
                                         start=(k == 0), stop=(k == 1))
                    zt = FW.tile([DOUT, 128], F32, tag="zt")
                    nc.scalar.activation(zt[:], zp[:], AF.Identity,
                                         bias=b2tot[:, 0:1])
                    zn = FPS.tile([128, DOUT], F32, tag="hp")
                    nc.tensor.matmul(zn[:], lhsT=zt[:], rhs=id64[:],
                                     start=True, stop=True)
                    nc.vector.tensor_scalar(
                        out=adz0[:, i * DOUT:(i + 1) * DOUT], in0=zn[:],
                        scalar1=adinv[:, i:i + 1], scalar2=None, op0=ALU.mult)
                    nc.vector.tensor_scalar(
                        out=carry[:, i * DOUT:(i + 1) * DOUT], in0=zn[:],
                        scalar1=dinv[:, i:i + 1], scalar2=None, op0=ALU.mult)

            # ---------------- propagation hops
            ET = mybir.EngineType
            with (
                tc.tile_pool(name="gat", bufs=8) as GP,
                tc.tile_pool(name="selp", bufs=6) as SP,
                tc.tile_pool(name="idxp", bufs=3) as IP,
                tc.tile_pool(name="updp", bufs=4) as UP,
                tc.tile_pool(name="aggp", bufs=4, space="PSUM") as APS,
            ):
                def publish_and_gather_hop(body_emit):
                    nc.gpsimd.dma_start(
                        out=agin_d[:, 0:DOUT].rearrange("(b p) f -> p b f",
                                                        p=128),
                        in_=carry[:].rearrange("p (b f) -> p b f", f=DOUT))
                    nc.gpsimd.collective_compute(
                        "AllGather", ALU.bypass, replica_groups=RG,
                        ins=[agin_d.ap().opt()], outs=[agfull_d.ap().opt()])
                    body_emit()

                def emit_superblock(ivx, lasthop):
                    """ivx: dynamic iv (RuntimeValue) or static int superblock."""
                    if STAGE == "pub":
                        return
                    mts = {}
                    for w in range(NWIN):
                        it = IP.tile([128, NI16U], I16, tag="idx", name="it")
                        nc.sync.dma_start(
                            out=it[:],
                            in_=idx_d[:, bass.ds(ivx * (NWIN * NI16U)
                                                 + w * NI16U, NI16U)])
                        mt = GP.tile([128, SBLK * CU * DOUT], BF16, tag="gt",
                                     name="mt")
                        nc.gpsimd.dma_gather(
                            mt[:].rearrange("p (c d) -> p c d", d=DOUT),
                            agfull_d[w * WROWS:(w + 1) * WROWS, 0:DOUT],
                            it[:], NIU, NIU, DOUT, elem_step=TW,
                            single_packet=False)
                        mts[w] = mt
                    if STAGE == "gather":
                        return
                    for i in range(SBLK):
                        aps = APS.tile([128, DOUT], F32, tag="agg", name="aps")
                        t = 0
                        nch_b = NWIN * CU
                        for w in range(NWIN):
                            for j in range(CU):
                                sel = SP.tile([128, 128], BF16, tag="sel",
                                              name="sel")
                                ccol = (ivx * (SBLK * NWIN * CU)
                                        + (i * NWIN + w) * CU + j)
                                nc.vector.tensor_scalar(
                                    out=sel[:], in0=iotab[:],
                                    scalar1=colloc[:, bass.ds(ccol, 1)],
                                    scalar2=None, op0=ALU.is_equal)
                                ci = i * CU + j
                                nc.tensor.matmul(
                                    aps[:], lhsT=sel[:],
                                    rhs=mts[w][:, ci * DOUT:(ci + 1) * DOUT],
                                    start=(t == 0), stop=(t == nch_b - 1))
                                t += 1
                        bb_ = ivx * SBLK + i
                        cslice = carry[:, bass.ds(bb_ * DOUT, DOUT)]
                        u0 = UP.tile([128, DOUT], F32, tag="u0", name="u0")
                        nc.vector.tensor_add(u0[:], aps[:], cslice)
                        if not lasthop:
                            nc.vector.scalar_tensor_tensor(
                                out=cslice, in0=u0[:],
                                scalar=dinv2s[:, bass.ds(bb_, 1)],
                                in1=adz0[:, bass.ds(bb_ * DOUT, DOUT)],
                                op0=ALU.mult, op1=ALU.add)
                        else:
                            b = bb_  # static int on the last hop
                            az = UP.tile([128, DOUT], F32, tag="az", name="az")
                            nc.vector.tensor_scalar(
                                out=az[:], in0=adz0[:, b * DOUT:(b + 1) * DOUT],
                                scalar1=sqdeg[:, b:b + 1],
                                scalar2=None, op0=ALU.mult)
                            ez = UP.tile([128, DOUT], F32, tag="ez", name="ez")
                            nc.vector.scalar_tensor_tensor(
                                out=ez[:], in0=u0[:],
                                scalar=dinv1s[:, b:b + 1], in1=az[:],
                                op0=ALU.mult, op1=ALU.add)
                            nc.sync.dma_start(
                                out=emb_d[b * 128:(b + 1) * 128, :], in_=ez[:])
                            mx = UP.tile([128, 1], F32, tag="mx", name="mx")
                            nc.vector.reduce_max(mx[:], ez[:], axis=AX.X)
                            nmx = UP.tile([128, 1], F32, tag="nmx", name="nmx")
                            nc.vector.tensor_scalar(
                                out=nmx[:], in0=mx[:], scalar1=-1.0,
                                scalar2=None, op0=ALU.mult)
                            exv = UP.tile([128, DOUT], F32, tag="exv",
                                          name="exv")
                            se = UP.tile([128, 1], F32, tag="se", name="se")
                            nc.scalar.activation(exv[:], ez[:], AF.Exp,
                                                 bias=nmx[:, 0:1],
                                                 accum_out=se[:])
                            ls = UP.tile([128, 1], F32, tag="ls", name="ls")
                            nc.scalar.activation(ls[:], se[:], AF.Ln)
                            mls = UP.tile([128, 1], F32, tag="mls", name="mls")
                            nc.vector.tensor_add(mls[:], mx[:], ls[:])
                            ot = UP.tile([128, DOUT], F32, tag="ot", name="ot")
                            nc.vector.tensor_scalar(
                                out=ot[:], in0=ez[:], scalar1=mls[:, 0:1],
                                scalar2=None, op0=ALU.subtract)
                            nc.sync.dma_start(
                                out=out_d[b * 128:(b + 1) * 128, :], in_=ot[:])

                if STAGE == "front":
                    hops_to_run = 0
                else:
                    hops_to_run = KHOPS - 1
                for hop in range(hops_to_run):
                    def body():
                        with tc.For_i(0, NSB, 1,
                                      hint_engines=(ET.PE, ET.DVE,
                                                    ET.Pool)) as iv:
                            emit_superblock(iv, False)
                    publish_and_gather_hop(body)

                if STAGE != "front":
                    def lastbody():
                        for sb in range(NSB):
                            emit_superblock(sb, True)
                    publish_and_gather_hop(lastbody)

    nc.compile()
    return nc


# ---------------------------------------------------------------- entry
def kernel(x, edge_index, W1, b1, gamma, beta, W2, b2):
    x = np.asarray(x, np.float32)
    edge_index = np.asarray(edge_index, np.int32)
    W1 = np.asarray(W1, np.float32)
    b1 = np.asarray(b1, np.float32)
    gamma = np.asarray(gamma, np.float32)
    beta = np.asarray(beta, np.float32)
    W2 = np.asarray(W2, np.float32)
    b2 = np.asarray(b2, np.float32)

    key = hashlib.sha1(edge_index.tobytes()).hexdigest()
    if key not in _CACHE:
        plan = _preprocess(edge_index)
        nc = _build(plan)
        _CACHE[key] = (plan, nc)
    plan, nc = _CACHE[key]

    iotab = np.tile(np.arange(128, dtype=np.float32), (128, 1)).astype(
        ml_dtypes.bfloat16)
    id128 = np.eye(128, dtype=np.float32)
    id64 = np.eye(64, dtype=np.float32)
    b1h = b1.reshape(2, 128).T.copy()
    gmh = gamma.reshape(2, 128).T.copy()
    bth = beta.reshape(2, 128).T.copy()
    b2c = b2.reshape(DOUT, 1).copy()

    in_maps = []
    for c in range(NCORES):
        xs = np.zeros((NPAD, DIN), np.float32)
        xs[:NLOC] = x[c * NLOC:(c + 1) * NLOC]
        in_maps.append({
            "x": xs,
            "w1": W1, "w2": W2, "b1h": b1h, "gammah": gmh, "betah": bth,
            "b2c": b2c,
            "degt": plan["deg_t"][c],
            "iotab": iotab, "id128": id128, "id64": id64,
            "idx": plan["idx_all"][c],
            "colloc": plan["colloc_all"][c],
        })

    res = run_bass_kernel_spmd(nc, in_maps, core_ids=list(range(NCORES)))
    out = np.concatenate([res.results[c]["out"][:NLOC] for c in range(NCORES)])
    emb = np.concatenate([res.results[c]["emb"][:NLOC] for c in range(NCORES)])
    return out, emb


# revision 12
# speedup vs baseline: 1.5970x; 1.5970x over previous
"""Distributed APPNP-stack kernel for one TRN2 chip (8 NeuronCores).

Pipeline: Linear(256->256) -> BatchNorm(batch stats) -> Linear(256->64)
-> 10 APPNP hops with GCN normalization -> log_softmax.

Sharding: nodes are partitioned contiguously across the 8 cores (12500
each). Each hop, every core publishes its shard of dinv*carry (bf16),
an AllGather makes the full table visible, and each core gathers the
source rows of the edges targeting its own nodes (dma_gather, int16
windows) and scatter-adds them with a one-hot-matmul into PSUM.

Host-side preprocessing is pure integer index work (CSR-style sort,
bucket/pad layout, degree counts). All float math runs on device.
"""

import hashlib
import inspect
import re

import ml_dtypes
import numpy as np

import concourse.bacc as bacc
import concourse.bass as bass
import concourse.mybir as mybir
from concourse.tile import TileContext
from concourse.bass_utils import run_bass_kernel_spmd

# ---------------------------------------------------------------- constants
N = 100000
E = 3200000
DIN = 256
HID = 256
DOUT = 64
KHOPS = 10
ALPHA = 0.1
BNEPS = 1e-5

NCORES = 8
NLOC = N // NCORES          # 12500 nodes owned per core
NB = (NLOC + 127) // 128    # 98 target blocks per core
NPAD = NB * 128             # 12544 padded local nodes
NREAL_LAST = NLOC - (NB - 1) * 128  # 84 real targets in the last block
NWIN = 4                    # int16 source windows (2 shards each)
WROWS = 2 * NPAD            # 25088 rows per window
SBLK = 7                    # target blocks per gather superblock
NSB = NB // SBLK            # 14
TW = 2 * DOUT               # padded table row width (256B bf16 stride)

F32 = mybir.dt.float32
BF16 = mybir.dt.bfloat16
I32 = mybir.dt.int32
I16 = mybir.dt.int16
AF = mybir.ActivationFunctionType
ALU = mybir.AluOpType
AX = mybir.AxisListType

_CACHE = {}
STAGE = "full"  # debug: front | pub | gather | full


def _patch_dma_gather():
    """Relax dma_gather's 256B elem-size assert to 128B (the real ucode
    constraint is the 256B *stride*, which we satisfy via elem_step)."""
    if getattr(bass.BassGpSimd.dma_gather, "_relaxed", False):
        return
    src = inspect.getsource(bass.BassGpSimd.dma_gather)
    src = src.replace("elem_size_bytes % 256 == 0", "elem_size_bytes % 128 == 0")
    src = "def dma_gather" + src.split("def dma_gather", 1)[1]
    src = re.sub(r"^    ", "", src, flags=re.M)
    ns = dict(bass.__dict__)
    exec(src, ns)
    ns["dma_gather"]._relaxed = True
    bass.BassGpSimd.dma_gather = ns["dma_gather"]


# ---------------------------------------------------------------- host prep
def _preprocess(edge_index):
    row = edge_index[0].astype(np.int64)
    col = edge_index[1].astype(np.int64)

    owner = col // NLOC
    lt = col - owner * NLOC
    blk = lt >> 7
    cloc = (lt & 127).astype(np.int32)
    gsrc = (row // NLOC) * NPAD + (row % NLOC)
    win = gsrc // WROWS
    lsrc = (gsrc - win * WROWS).astype(np.int32)

    key = (owner * NB + blk) * NWIN + win
    counts = np.bincount(key, minlength=NCORES * NB * NWIN).reshape(
        NCORES, NB, NWIN
    )
    order = np.argsort(key, kind="stable")
    lsrc_s = lsrc[order]
    cloc_s = cloc[order]
    starts = np.zeros(NCORES * NB * NWIN + 1, np.int64)
    np.cumsum(counts.ravel(), out=starts[1:])

    # uniform chunk count per (block, window): identical loop body on all
    # cores and all superblocks (required for the For_i sem-reset structure)
    CU = int(-(-counts.max() // 128))
    TOTC = NB * NWIN * CU
    NI16U = 8 * SBLK * CU                      # idx cols per (sb, w)
    TOT16 = NSB * NWIN * NI16U

    # per-core tables
    idx_all = np.zeros((NCORES, 128, TOT16), np.int16)
    colloc_all = np.full((NCORES, 128, TOTC), -1.0, np.float32)
    for c in range(NCORES):
        for b in range(NB):
            for w in range(NWIN):
                s0 = starts[(c * NB + b) * NWIN + w]
                s1 = starts[(c * NB + b) * NWIN + w + 1]
                cnt = int(s1 - s0)
                flatc = np.full(CU * 128, -1.0, np.float32)
                flatc[:cnt] = cloc_s[s0:s1]
                gcol = (b * NWIN + w) * CU
                colloc_all[c, :, gcol : gcol + CU] = flatc.reshape(CU, 128).T
        for sb in range(NSB):
            for w in range(NWIN):
                ni = SBLK * CU * 128
                flat = np.zeros(ni, np.int16)
                for i, b in enumerate(range(sb * SBLK, (sb + 1) * SBLK)):
                    s0 = starts[(c * NB + b) * NWIN + w]
                    s1 = starts[(c * NB + b) * NWIN + w + 1]
                    off = i * CU * 128
                    flat[off : off + (s1 - s0)] = lsrc_s[s0:s1]
                wrapped = flat.reshape(-1, 16).T          # [16, ni/16]
                c0 = (sb * NWIN + w) * NI16U
                idx_all[c, :, c0 : c0 + ni // 16] = np.tile(wrapped, (8, 1))

    deg = np.bincount(col, minlength=N).astype(np.int32) + 1
    deg_t = np.ones((NCORES, 128, NB), np.int32)
    for c in range(NCORES):
        dl = np.ones(NPAD, np.int32)
        dl[:NLOC] = deg[c * NLOC : (c + 1) * NLOC]
        deg_t[c] = dl.reshape(NB, 128).T

    return dict(
        CU=CU, TOTC=TOTC, TOT16=TOT16, NI16U=NI16U,
        idx_all=idx_all, colloc_all=colloc_all, deg_t=deg_t,
    )


# ---------------------------------------------------------------- program
def _build(plan):
    _patch_dma_gather()
    CU = plan["CU"]
    TOTC = plan["TOTC"]
    TOT16 = plan["TOT16"]
    NI16U = plan["NI16U"]
    NIU = SBLK * CU * 128                      # idxs per (sb, w) gather
    RG = [list(range(NCORES))]

    nc = bacc.Bacc("TRN2", debug=False, num_devices=NCORES,
                   num_swdge_queues=4)

    x_d = nc.dram_tensor("x", [NPAD, DIN], F32, kind="ExternalInput")
    w1_d = nc.dram_tensor("w1", [DIN, HID], F32, kind="ExternalInput")
    w2_d = nc.dram_tensor("w2", [HID, DOUT], F32, kind="ExternalInput")
    b1_d = nc.dram_tensor("b1h", [128, 2], F32, kind="ExternalInput")
    gm_d = nc.dram_tensor("gammah", [128, 2], F32, kind="ExternalInput")
    bt_d = nc.dram_tensor("betah", [128, 2], F32, kind="ExternalInput")
    b2_d = nc.dram_tensor("b2c", [DOUT, 1], F32, kind="ExternalInput")
    deg_d = nc.dram_tensor("degt", [128, NB], I32, kind="ExternalInput")
    iota_d = nc.dram_tensor("iotab", [128, 128], BF16, kind="ExternalInput")
    iotaw_d = nc.dram_tensor("iotaw", [128, NWIN * CU * 128], BF16,
                             kind="ExternalInput")
    id128_d = nc.dram_tensor("id128", [128, 128], F32, kind="ExternalInput")
    id64_d = nc.dram_tensor("id64", [64, 64], F32, kind="ExternalInput")
    idx_d = nc.dram_tensor("idx", [128, TOT16], I16, kind="ExternalInput")
    cl_d = nc.dram_tensor("colloc", [128, TOTC], F32, kind="ExternalInput")
    out_d = nc.dram_tensor("out", [NPAD, DOUT], F32, kind="ExternalOutput")
    emb_d = nc.dram_tensor("emb", [NPAD, DOUT], F32, kind="ExternalOutput")

    agin_d = nc.dram_tensor("agin", [NPAD, TW], BF16)
    agfull_d = nc.dram_tensor("agfull", [NCORES * NPAD, TW], BF16,
                              addr_space="Shared")
    stin_d = nc.dram_tensor("stin", [128, 4], F32)
    stout_d = nc.dram_tensor("stout", [128, 4], F32, addr_space="Shared")

    with TileContext(nc) as tc:
        with (
            tc.tile_pool(name="persist", bufs=1) as PP,
            tc.tile_pool(name="ppsum", bufs=2, space="PSUM") as PPS,
        ):
            # ---------------- persistent tiles
            carry = PP.tile([128, NB * DOUT], BF16)
            adz0 = PP.tile([128, NB * DOUT], F32)
            colloc = PP.tile([128, TOTC], F32)
            iotab = PP.tile([128, 128], BF16)
            iotaw = PP.tile([128, NWIN * CU * 128], BF16)
            id128 = PP.tile([128, 128], F32)
            id64 = PP.tile([64, 64], F32)
            w1k = [PP.tile([128, HID], F32, tag=f"w1_{k}", name=f"w1_{k}") for k in range(2)]
            w2k = [PP.tile([128, DOUT], F32, tag=f"w2_{k}", name=f"w2_{k}") for k in range(2)]
            w2p = [PP.tile([128, DOUT], F32, tag=f"w2p_{k}", name=f"w2p_{k}") for k in range(2)]
            b1t = PP.tile([128, 2], F32)
            gmt = PP.tile([128, 2], F32)
            btt = PP.tile([128, 2], F32)
            b2c = PP.tile([DOUT, 1], F32)
            b2tot = PP.tile([DOUT, 1], F32)
            degf = PP.tile([128, NB], F32)
            sqdeg = PP.tile([128, NB], F32)
            dinv = PP.tile([128, NB], F32)
            adinv = PP.tile([128, NB], F32)
            dinv1s = PP.tile([128, NB], F32)
            dinv2s = PP.tile([128, NB], F32)
            ones11 = PP.tile([1, 1], F32)
            epsb = PP.tile([128, 1], F32)
            hs = [PP.tile([128, NB], F32, tag=f"hs{m}", name=f"hs{m}") for m in range(2)]
            hq = [PP.tile([128, NB], F32, tag=f"hq{m}", name=f"hq{m}") for m in range(2)]
            stats = PP.tile([128, 4], F32)
            statsg = PP.tile([128, 4], F32)

            nc.sync.dma_start(out=colloc[:], in_=cl_d[:, :])
            nc.sync.dma_start(out=iotab[:], in_=iota_d[:, :])
            nc.sync.dma_start(out=iotaw[:], in_=iotaw_d[:, :])
            nc.sync.dma_start(out=id128[:], in_=id128_d[:, :])
            nc.sync.dma_start(out=id64[:], in_=id64_d[:, :])
            nc.sync.dma_start(out=w1k[0][:], in_=w1_d[0:128, :])
            nc.sync.dma_start(out=w1k[1][:], in_=w1_d[128:256, :])
            nc.sync.dma_start(out=w2k[0][:], in_=w2_d[0:128, :])
            nc.sync.dma_start(out=w2k[1][:], in_=w2_d[128:256, :])
            nc.sync.dma_start(out=b1t[:], in_=b1_d[:, :])
            nc.sync.dma_start(out=gmt[:], in_=gm_d[:, :])
            nc.sync.dma_start(out=btt[:], in_=bt_d[:, :])
            nc.sync.dma_start(out=b2c[:], in_=b2_d[:, :])
            dg_i = PP.tile([128, NB], I32)
            nc.sync.dma_start(out=dg_i[:], in_=deg_d[:, :])
            nc.vector.memset(ones11[:], 1.0)
            nc.vector.memset(epsb[:], BNEPS)

            # degree-derived per-node scalars
            nc.vector.tensor_copy(out=degf[:], in_=dg_i[:])
            nc.scalar.sqrt(sqdeg[:], degf[:])
            nc.vector.reciprocal(dinv[:], sqdeg[:])
            nc.vector.tensor_scalar(out=adinv[:], in0=dinv[:], scalar1=ALPHA,
                                    scalar2=None, op0=ALU.mult)
            nc.vector.tensor_scalar(out=dinv1s[:], in0=dinv[:],
                                    scalar1=1.0 - ALPHA, scalar2=None,
                                    op0=ALU.mult)
            nc.vector.tensor_mul(dinv2s[:], dinv[:], dinv1s[:])

            # ---------------- front: x @ W1 (+stats) -> BN fold -> @W2
            with (
                tc.tile_pool(name="fbig", bufs=1) as FB,
                tc.tile_pool(name="fwork", bufs=3) as FW,
                tc.tile_pool(name="fpsum", bufs=2, space="PSUM") as FPS,
            ):
                hT = [FB.tile([128, NPAD], F32, tag=f"hT{m}", name=f"hT{m}") for m in range(2)]
                for i in range(NB):
                    xt = FW.tile([128, DIN], F32, tag="xt")
                    nc.sync.dma_start(out=xt[:], in_=x_d[i * 128:(i + 1) * 128, :])
                    xT = []
                    for k in range(2):
                        tp = FPS.tile([128, 128], F32, tag="tp")
                        nc.tensor.transpose(out=tp[:],
                                            in_=xt[:, k * 128:(k + 1) * 128],
                                            identity=id128[:])
                        xs = FW.tile([128, 128], F32, tag=f"xT{k}")
                        nc.vector.tensor_copy(out=xs[:], in_=tp[:])
                        xT.append(xs)
                    for m in range(2):
                        hp = FPS.tile([128, 128], F32, tag="hp")
                        for k in range(2):
                            nc.tensor.matmul(hp[:],
                                             lhsT=w1k[k][:, m * 128:(m + 1) * 128],
                                             rhs=xT[k][:],
                                             start=(k == 0), stop=(k == 1))
                        dst = hT[m][:, i * 128:(i + 1) * 128]
                        if i < NB - 1:
                            nc.scalar.activation(dst, hp[:], AF.Identity,
                                                 bias=b1t[:, m:m + 1],
                                                 accum_out=hs[m][:, i:i + 1])
                            sqs = FW.tile([128, 128], F32, tag="sqs")
                            nc.scalar.activation(sqs[:], dst, AF.Square,
                                                 accum_out=hq[m][:, i:i + 1])
                        else:
                            nr = NREAL_LAST
                            nc.scalar.activation(dst[:, :nr], hp[:, :nr],
                                                 AF.Identity,
                                                 bias=b1t[:, m:m + 1],
                                                 accum_out=hs[m][:, i:i + 1])
                            sqs = FW.tile([128, 128], F32, tag="sqs")
                            nc.scalar.activation(sqs[:, :nr], dst[:, :nr],
                                                 AF.Square,
                                                 accum_out=hq[m][:, i:i + 1])
                            nc.vector.tensor_copy(out=dst[:, nr:],
                                                  in_=hp[:, nr:])

                # global batch stats
                nc.vector.reduce_sum(stats[:, 0:1], hs[0][:], axis=AX.X)
                nc.vector.reduce_sum(stats[:, 1:2], hs[1][:], axis=AX.X)
                nc.vector.reduce_sum(stats[:, 2:3], hq[0][:], axis=AX.X)
                nc.vector.reduce_sum(stats[:, 3:4], hq[1][:], axis=AX.X)
                nc.sync.dma_start(out=stin_d[:, :], in_=stats[:])
                nc.gpsimd.collective_compute(
                    "AllReduce", ALU.add, replica_groups=RG,
                    ins=[stin_d.ap().opt()], outs=[stout_d.ap().opt()])
                nc.sync.dma_start(out=statsg[:], in_=stout_d[:, :])

                mean = FW.tile([128, 2], F32, tag="mean")
                ex2 = FW.tile([128, 2], F32, tag="ex2")
                var = FW.tile([128, 2], F32, tag="var")
                sd = FW.tile([128, 2], F32, tag="sd")
                sinv = FW.tile([128, 2], F32, tag="sinv")
                aa = FW.tile([128, 2], F32, tag="aa")
                cc = FW.tile([128, 2], F32, tag="cc")
                nc.vector.tensor_scalar(out=mean[:], in0=statsg[:, 0:2],
                                        scalar1=1.0 / N, scalar2=None,
                                        op0=ALU.mult)
                nc.vector.tensor_scalar(out=ex2[:], in0=statsg[:, 2:4],
                                        scalar1=1.0 / N, scalar2=None,
                                        op0=ALU.mult)
                msq = FW.tile([128, 2], F32, tag="msq")
                nc.vector.tensor_mul(msq[:], mean[:], mean[:])
                nc.vector.tensor_sub(var[:], ex2[:], msq[:])
                nc.scalar.activation(sd[:], var[:], AF.Sqrt, bias=epsb[:, 0:1])
                nc.vector.reciprocal(sinv[:], sd[:])
                nc.vector.tensor_mul(aa[:], sinv[:], gmt[:])
                mt_ = FW.tile([128, 2], F32, tag="mt_")
                nc.vector.tensor_mul(mt_[:], mean[:], aa[:])
                nc.vector.tensor_sub(cc[:], btt[:], mt_[:])
                for k in range(2):
                    nc.vector.tensor_scalar(out=w2p[k][:], in0=w2k[k][:],
                                            scalar1=aa[:, k:k + 1],
                                            scalar2=None, op0=ALU.mult)
                bb = FPS.tile([1, DOUT], F32, tag="tp")
                for k in range(2):
                    nc.tensor.matmul(bb[:], lhsT=cc[:, k:k + 1], rhs=w2k[k][:],
                                     start=(k == 0), stop=(k == 1))
                btmp = FW.tile([1, DOUT], F32, tag="btmp")
                nc.vector.tensor_copy(out=btmp[:], in_=bb[:])
                bcp = FPS.tile([DOUT, 1], F32, tag="hp")
                nc.tensor.matmul(bcp[:], lhsT=btmp[:], rhs=ones11[:],
                                 start=True, stop=True)
                nc.vector.tensor_add(b2tot[:], bcp[:], b2c[:])

                # z = hBN @ W2' ; store carry'_0 and alpha*dinv*z0
                for i in range(NB):
                    zp = FPS.tile([DOUT, 128], F32, tag="tp")
                    for k in range(2):
                        nc.tensor.matmul(zp[:], lhsT=w2p[k][:],
                                         rhs=hT[k][:, i * 128:(i + 1) * 128],
                                         start=(k == 0), stop=(k == 1))
                    zt = FW.tile([DOUT, 128], F32, tag="zt")
                    nc.scalar.activation(zt[:], zp[:], AF.Identity,
                                         bias=b2tot[:, 0:1])
                    zn = FPS.tile([128, DOUT], F32, tag="hp")
                    nc.tensor.matmul(zn[:], lhsT=zt[:], rhs=id64[:],
                                     start=True, stop=True)
                    nc.vector.tensor_scalar(
                        out=adz0[:, i * DOUT:(i + 1) * DOUT], in0=zn[:],
                        scalar1=adinv[:, i:i + 1], scalar2=None, op0=ALU.mult)
                    nc.vector.tensor_scalar(
                        out=carry[:, i * DOUT:(i + 1) * DOUT], in0=zn[:],
                        scalar1=dinv[:, i:i + 1], scalar2=None, op0=ALU.mult)

            # ---------------- propagation hops
            ET = mybir.EngineType
            with (
                tc.tile_pool(name="gat", bufs=8) as GP,
                tc.tile_pool(name="selp", bufs=6) as SP,
                tc.tile_pool(name="idxp", bufs=3) as IP,
                tc.tile_pool(name="updp", bufs=4) as UP,
                tc.tile_pool(name="aggp", bufs=4, space="PSUM") as APS,
            ):
                def publish_and_gather_hop(body_emit):
                    nc.sync.dma_start(
                        out=agin_d[:, 0:DOUT].rearrange("(b p) f -> p b f",
                                                        p=128),
                        in_=carry[:].rearrange("p (b f) -> p b f", f=DOUT))
                    nc.gpsimd.collective_compute(
                        "AllGather", ALU.bypass, replica_groups=RG,
                        ins=[agin_d.ap().opt()], outs=[agfull_d.ap().opt()])
                    body_emit()

                def emit_superblock(ivx, lasthop):
                    """ivx: dynamic iv (RuntimeValue) or static int superblock."""
                    if STAGE == "pub":
                        return
                    mts = {}
                    for w in range(NWIN):
                        it = IP.tile([128, NI16U], I16, tag="idx", name="it")
                        nc.sync.dma_start(
                            out=it[:],
                            in_=idx_d[:, bass.ds(ivx * (NWIN * NI16U)
                                                 + w * NI16U, NI16U)])
                        mt = GP.tile([128, SBLK * CU * DOUT], BF16, tag="gt",
                                     name="mt")
                        nc.gpsimd.dma_gather(
                            mt[:].rearrange("p (c d) -> p c d", d=DOUT),
                            agfull_d[w * WROWS:(w + 1) * WROWS, 0:DOUT],
                            it[:], NIU, NIU, DOUT, elem_step=TW,
                            single_packet=False, queue_num=w)
                        mts[w] = mt
                    if STAGE == "gather":
                        return
                    CS = SBLK * NWIN * CU
                    ctile = IP.tile([128, CS], F32, tag="ct", name="ctile")
                    nc.scalar.dma_start(
                        out=ctile[:], in_=cl_d[:, bass.ds(ivx * CS, CS)])
                    for i in range(SBLK):
                        aps = APS.tile([128, DOUT], F32, tag="agg", name="aps")
                        nch_b = NWIN * CU
                        selw = SP.tile([128, nch_b * 128], BF16, tag="sel",
                                       name="selw")
                        cslice = ctile[:, i * nch_b:(i + 1) * nch_b]
                        nc.vector.tensor_tensor(
                            out=selw[:].rearrange("p (c t) -> p c t", t=128),
                            in0=iotaw[:].rearrange("p (c t) -> p c t", t=128),
                            in1=cslice[:, :, None].to_broadcast(
                                [128, nch_b, 128]),
                            op=ALU.is_equal)
                        t = 0
                        for w in range(NWIN):
                            for j in range(CU):
                                ci = i * CU + j
                                tt = w * CU + j
                                nc.tensor.matmul(
                                    aps[:], lhsT=selw[:, tt * 128:(tt + 1) * 128],
                                    rhs=mts[w][:, ci * DOUT:(ci + 1) * DOUT],
                                    start=(t == 0), stop=(t == nch_b - 1))
                                t += 1
                        bb_ = ivx * SBLK + i
                        cslice = carry[:, bass.ds(bb_ * DOUT, DOUT)]
                        u0 = UP.tile([128, DOUT], F32, tag="u0", name="u0")
                        nc.vector.tensor_add(u0[:], aps[:], cslice)
                        if not lasthop:
                            nc.vector.scalar_tensor_tensor(
                                out=cslice, in0=u0[:],
                                scalar=dinv2s[:, bass.ds(bb_, 1)],
                                in1=adz0[:, bass.ds(bb_ * DOUT, DOUT)],
                                op0=ALU.mult, op1=ALU.add)
                        else:
                            b = bb_  # static int on the last hop
                            az = UP.tile([128, DOUT], F32, tag="az", name="az")
                            nc.vector.tensor_scalar(
                                out=az[:], in0=adz0[:, b * DOUT:(b + 1) * DOUT],
                                scalar1=sqdeg[:, b:b + 1],
                                scalar2=None, op0=ALU.mult)
                            ez = UP.tile([128, DOUT], F32, tag="ez", name="ez")
                            nc.vector.scalar_tensor_tensor(
                                out=ez[:], in0=u0[:],
                                scalar=dinv1s[:, b:b + 1], in1=az[:],
                                op0=ALU.mult, op1=ALU.add)
                            nc.sync.dma_start(
                                out=emb_d[b * 128:(b + 1) * 128, :], in_=ez[:])
                            mx = UP.tile([128, 1], F32, tag="mx", name="mx")
                            nc.vector.reduce_max(mx[:], ez[:], axis=AX.X)
                            nmx = UP.tile([128, 1], F32, tag="nmx", name="nmx")
                            nc.vector.tensor_scalar(
                                out=nmx[:], in0=mx[:], scalar1=-1.0,
                                scalar2=None, op0=ALU.mult)
                            exv = UP.tile([128, DOUT], F32, tag="exv",
                                          name="exv")
                            se = UP.tile([128, 1], F32, tag="se", name="se")
                            nc.scalar.activation(exv[:], ez[:], AF.Exp,
                                                 bias=nmx[:, 0:1],
                                                 accum_out=se[:])
                            ls = UP.tile([128, 1], F32, tag="ls", name="ls")
                            nc.scalar.activation(ls[:], se[:], AF.Ln)
                            mls = UP.tile([128, 1], F32, tag="mls", name="mls")
                            nc.vector.tensor_add(mls[:], mx[:], ls[:])
                            ot = UP.tile([128, DOUT], F32, tag="ot", name="ot")
                            nc.vector.tensor_scalar(
                                out=ot[:], in0=ez[:], scalar1=mls[:, 0:1],
                                scalar2=None, op0=ALU.subtract)
                            nc.sync.dma_start(
                                out=out_d[b * 128:(b + 1) * 128, :], in_=ot[:])

                if STAGE == "front":
                    hops_to_run = 0
                else:
                    hops_to_run = KHOPS - 1
                for hop in range(hops_to_run):
                    def body():
                        with tc.For_i(0, NSB, 1,
                                      hint_engines=(ET.PE, ET.DVE,
                                                    ET.Pool)) as iv:
                            emit_superblock(iv, False)
                    publish_and_gather_hop(body)

                if STAGE != "front":
                    def lastbody():
                        for sb in range(NSB):
                            emit_superblock(sb, True)
                    publish_and_gather_hop(lastbody)

    nc.compile()
    return nc



def _make_in_maps(plan, x, W1, b1, gamma, beta, W2, b2):
    iotab = np.tile(np.arange(128, dtype=np.float32), (128, 1)).astype(
        ml_dtypes.bfloat16)
    nchb = NWIN * plan["CU"]
    iotaw = np.tile(np.arange(128, dtype=np.float32)[None, None, :],
                    (128, nchb, 1)).astype(ml_dtypes.bfloat16).reshape(
        128, nchb * 128)
    id128 = np.eye(128, dtype=np.float32)
    id64 = np.eye(64, dtype=np.float32)
    b1h = np.asarray(b1, np.float32).reshape(2, 128).T.copy()
    gmh = np.asarray(gamma, np.float32).reshape(2, 128).T.copy()
    bth = np.asarray(beta, np.float32).reshape(2, 128).T.copy()
    b2c = np.asarray(b2, np.float32).reshape(DOUT, 1).copy()
    x = np.asarray(x, np.float32)
    in_maps = []
    for c in range(NCORES):
        xs = np.zeros((NPAD, DIN), np.float32)
        xs[:NLOC] = x[c * NLOC:(c + 1) * NLOC]
        in_maps.append({
            "x": xs,
            "w1": np.asarray(W1, np.float32), "w2": np.asarray(W2, np.float32),
            "b1h": b1h, "gammah": gmh, "betah": bth, "b2c": b2c,
            "degt": plan["deg_t"][c],
            "iotab": iotab, "iotaw": iotaw, "id128": id128, "id64": id64,
            "idx": plan["idx_all"][c],
            "colloc": plan["colloc_all"][c],
        })
    return in_maps


# ---------------------------------------------------------------- entry
def kernel(x, edge_index, W1, b1, gamma, beta, W2, b2):
    x = np.asarray(x, np.float32)
    edge_index = np.asarray(edge_index, np.int32)
    W1 = np.asarray(W1, np.float32)
    b1 = np.asarray(b1, np.float32)
    gamma = np.asarray(gamma, np.float32)
    beta = np.asarray(beta, np.float32)
    W2 = np.asarray(W2, np.float32)
    b2 = np.asarray(b2, np.float32)

    key = hashlib.sha1(edge_index.tobytes()).hexdigest()
    if key not in _CACHE:
        plan = _preprocess(edge_index)
        nc = _build(plan)
        _CACHE[key] = (plan, nc)
    plan, nc = _CACHE[key]

    in_maps = _make_in_maps(plan, x, W1, b1, gamma, beta, W2, b2)
    res = run_bass_kernel_spmd(nc, in_maps, core_ids=list(range(NCORES)))
    out = np.concatenate([res.results[c]["out"][:NLOC] for c in range(NCORES)])
    emb = np.concatenate([res.results[c]["emb"][:NLOC] for c in range(NCORES)])
    return out, emb
